# revision 4
# baseline (speedup 1.0000x reference)
# Trainium2 Bass kernel for the MEGNet edge model:
#   out = relu(concat([src, dest, edge_attr, u[batch]], 1) @ W1 + b1) @ W2 + b2
#
# Strategy (8 NeuronCores, SPMD, edges sharded contiguously):
#  * All tensors are shipped in a transposed, feature-major layout [128, E_pad]
#    so the PE array contracts over features with no on-chip transposes; the
#    host transposes shards on the way in and the output on the way out.
#  * comb @ W1 decomposes into src@W1a + dest@W1b + edge_attr@W1c +
#    u[batch]@W1d.  The u[batch] term plus b1 folds into a per-group table
#    z = u @ W1d + b1 [G, 128]; batch is sorted, so each 512-edge tile spans
#    only k_s consecutive groups (k_s=2 here) and z[batch] is applied with one
#    tiny K=k_s matmul per tile (lhsT = the candidate z-rows, rhs = a one-hot
#    selection matrix built on the host).
#  * Traffic is the roofline: src/dest/edge_attr/sel ship as fp8e3 (e3m4's 4
#    mantissa bits keep total rel err ~1.2e-2 vs the 2e-2 gate), output as
#    bf16.  ~32.5 MB/core total vs ~358 GB/s HBM-per-core.
#  * PE stream: 4-tile accumulation groups (4 p1 + 4 p2 PSUM banks).  The
#    three W1 matmuls are weight-stationary runs of 4; the four selection
#    matmuls run CONCURRENTLY on the four 32-row strips of the PE array
#    (tile_position=(32j,0), j=tile%4) so they cost ~1 matmul slot instead of
#    4; the W2 flushes of group g-1 run as one stationary-weight run of 4 in
#    the middle of group g, so the PE never waits on a ReLU.  Input streams
#    ride the sync hardware-DGE queue; outputs ride gpsimd's software-DGE
#    queue, pre-warmed by a dummy transfer.
import os
import numpy as np

N_CORES = 8
P = 128      # feature dim == SBUF partitions
TILE = 512   # edges per matmul tile (one PSUM bank of fp32)
CH = 14      # max matmul tiles per DMA chunk (pool slot size)
# chunk sizes: smaller leading chunks let compute start sooner, and a small
# final chunk keeps the output-DMA drain tail short
CHUNK_SIZES = [10] + [14] * 6 + [4]                    # sums to 98 tiles
GRP = 4      # tiles per PE accumulation group (= row strips for sel matmuls)
OG = 8       # tiles per output DMA group

_prog_cache = {}


def _build_program(T, k_s):
    import concourse.bacc as bacc
    import concourse.tile as tile
    from concourse import mybir

    f32 = mybir.dt.float32
    mdt = mybir.dt.bfloat16
    sdt = mybir.dt.float8e3
    odt = mybir.dt.bfloat16
    Relu = mybir.ActivationFunctionType.Relu
    Epad = T * TILE
    assert k_s <= 32

    nc = bacc.Bacc("TRN2", target_bir_lowering=False, debug=False,
                   num_devices=N_CORES)
    srcT = nc.dram_tensor("srcT", [P, Epad], sdt, kind="ExternalInput")
    destT = nc.dram_tensor("destT", [P, Epad], sdt, kind="ExternalInput")
    eaT = nc.dram_tensor("eaT", [P, Epad], sdt, kind="ExternalInput")
    wpkd = nc.dram_tensor("wpk", [P, 4 * P], mdt, kind="ExternalInput")
    b2d = nc.dram_tensor("b2c", [P, 1], f32, kind="ExternalInput")
    # sel rows / z-rows for tile t live at row-strip t%GRP (partitions
    # 32*(t%GRP) .. +k_s) so GRP consecutive tiles' selection matmuls can run
    # concurrently on distinct 32-row strips of the PE array.
    seld = nc.dram_tensor("sel", [GRP * k_s, Epad], sdt, kind="ExternalInput")
    zwd = nc.dram_tensor("zw", [GRP * k_s, T * P], mdt, kind="ExternalInput")
    outT = nc.dram_tensor("outT", [P, Epad], odt, kind="ExternalOutput")

    assert sum(CHUNK_SIZES) == T
    CW = CH * TILE  # max chunk width in edges (pool slot size)
    SELP = 32 * (GRP - 1) + k_s  # partitions needed by the strip layout

    with tile.TileContext(nc) as tc:
        with (
            tc.tile_pool(name="const", bufs=1) as constp,
            tc.tile_pool(name="inp", bufs=3) as inp,
            tc.tile_pool(name="hp", bufs=12) as hp,
            tc.tile_pool(name="outp", bufs=4) as outp,
            tc.tile_pool(name="ps1", bufs=GRP, space="PSUM") as ps1,
            tc.tile_pool(name="ps2", bufs=GRP, space="PSUM") as ps2,
        ):
            wpk = constp.tile([P, 4 * P], mdt, tag="wpk", name="wpk")
            b2s = constp.tile([P, 1], f32, tag="b2s", name="b2s")
            b2w = constp.tile([P, 1], f32, tag="b2w", name="b2w")
            zws = constp.tile([SELP, T * P], mdt, tag="zws", name="zws")
            nc.sync.dma_start(wpk[:], wpkd[:])
            # dummy early transfer to spin up the gpsimd software-DGE queue
            # (it takes ~10us from first use to first delivered packet; the
            # output tiles that ride it are first ready at ~20us)
            nc.gpsimd.dma_start(b2w[:], b2d[:])
            w1a = wpk[:, 0:P]
            w1b = wpk[:, P:2 * P]
            w1c = wpk[:, 2 * P:3 * P]
            w2s = wpk[:, 3 * P:4 * P]

            pend = []  # [(h_tile, tile_idx)] of the previous group
            ot_cur = [None, 0]  # current output tile, its base tile idx

            def flush_group():
                # One stationary W2 load feeds all pending flushes; the
                # vector add + output DMA trail each flush matmul.
                for h, ti in pend:
                    p2 = ps2.tile([P, TILE], f32, tag="p2", name=f"p2_{ti}")
                    nc.tensor.matmul(p2[:], w2s, h[:], start=True, stop=True)
                    og = ti // OG
                    o0 = og * OG
                    ow = (min(o0 + OG, T) - o0) * TILE
                    if ot_cur[0] is None or ot_cur[1] != o0:
                        ot_cur[0] = outp.tile([P, OG * TILE], odt, tag="o",
                                              name=f"ot{og}")
                        ot_cur[1] = o0
                    ot = ot_cur[0]
                    ocs = slice((ti - o0) * TILE, (ti - o0 + 1) * TILE)
                    nc.vector.tensor_scalar_add(ot[:, ocs], p2[:], b2s[:])
                    if ti == min(o0 + OG, T) - 1:
                        nc.gpsimd.dma_start(
                            outT[:, o0 * TILE:o0 * TILE + ow], ot[:, :ow])
                pend.clear()

            def emit_group(tiles):
                p1s = [ps1.tile([P, TILE], f32, tag="p1", name=f"p1_{ti}")
                       for ti, _, _ in tiles]
                for i, (ti, bufs, cs) in enumerate(tiles):
                    nc.tensor.matmul(p1s[i][:], w1a, bufs[0][:, cs],
                                     start=True, stop=False)
                for i, (ti, bufs, cs) in enumerate(tiles):
                    nc.tensor.matmul(p1s[i][:], w1b, bufs[1][:, cs],
                                     start=False, stop=False)
                # flush the previous group's W2 matmuls here: by now their
                # ReLUs have long finished, and the sel phase below gives the
                # scalar engine time to produce this group's h tiles
                flush_group()
                for i, (ti, bufs, cs) in enumerate(tiles):
                    nc.tensor.matmul(p1s[i][:], w1c, bufs[2][:, cs],
                                     start=False, stop=False)
                for i, (ti, bufs, cs) in enumerate(tiles):
                    j = ti % GRP
                    nc.tensor.matmul(p1s[i][:],
                                     zws[32 * j:32 * j + k_s,
                                         ti * P:(ti + 1) * P],
                                     bufs[3][32 * j:32 * j + k_s, cs],
                                     start=False, stop=True,
                                     tile_position=(32 * j, 0))
                    h = hp.tile([P, TILE], mdt, tag="h", name=f"h{ti}")
                    nc.scalar.activation(h[:], p1s[i][:], Relu)
                    pend.append((h, ti))

            t = 0
            tile_q = []
            for ci, csz in enumerate(CHUNK_SIZES):
                base = t * TILE
                cw = csz * TILE
                st = inp.tile([P, CW], sdt, tag="src", name=f"st{ci}")
                dt = inp.tile([P, CW], sdt, tag="dest", name=f"dt{ci}")
                et = inp.tile([P, CW], sdt, tag="ea", name=f"et{ci}")
                slt = inp.tile([SELP, CW], sdt, tag="sel", name=f"slt{ci}")
                # every input stream rides the sync hardware-DGE queue: sync
                # has no per-tile compute, so trigger instructions never gate
                # the ReLU/add pipeline on scalar/vector
                nc.sync.dma_start(st[:, :cw], srcT[:, base:base + cw])
                nc.sync.dma_start(dt[:, :cw], destT[:, base:base + cw])
                nc.sync.dma_start(et[:, :cw], eaT[:, base:base + cw])
                for j in range(GRP):
                    nc.sync.dma_start(
                        slt[32 * j:32 * j + k_s, :cw],
                        seld[k_s * j:k_s * (j + 1), base:base + cw])
                if ci == 0:  # constants queued behind chunk 0's streams
                    for j in range(GRP):
                        nc.sync.dma_start(
                            zws[32 * j:32 * j + k_s, :],
                            zwd[k_s * j:k_s * (j + 1), :])
                    nc.sync.dma_start(b2s[:], b2d[:])

                for tl in range(csz):
                    cs = slice(tl * TILE, (tl + 1) * TILE)
                    tile_q.append((t, (st, dt, et, slt), cs))
                    t += 1
                while len(tile_q) >= GRP:
                    emit_group(tile_q[:GRP])
                    del tile_q[:GRP]
            if tile_q:
                emit_group(tile_q)
            flush_group()

    nc.compile()
    return nc


def _get_program(T, k_s):
    key = (T, k_s)
    if key not in _prog_cache:
        _prog_cache[key] = _build_program(T, k_s)
    return _prog_cache[key]


def _install_profile_shim():
    """Optional: enable NTFF profiling under axon (KERNEL_PROFILE=1)."""
    import sys, types
    if "antenv.axon_hooks" not in sys.modules:
        mod = types.ModuleType("antenv.axon_hooks")
        mod._hook = None
        mod.set_axon_ntff_profile_hook = lambda h: setattr(mod, "_hook", h)
        mod.get_axon_ntff_profile_hook = lambda: mod._hook
        sys.modules["antenv.axon_hooks"] = mod
        try:
            import antenv
            antenv.axon_hooks = mod
        except ImportError:
            pass
        try:
            from trn_agent_boot.trn_boot import _ntff_profile_via_ctypes
            mod.set_axon_ntff_profile_hook(
                _ntff_profile_via_ctypes("/opt/axon/libaxon_pjrt.so"))
        except Exception:
            pass
    import concourse.bass_utils as bass_utils
    bass_utils.upload_artifacts = lambda tmpdir: tmpdir


def kernel(src, dest, edge_attr, u, batch, W1, b1, W2, b2):
    import ml_dtypes
    bf16 = ml_dtypes.bfloat16
    e3m4 = ml_dtypes.float8_e3m4

    src = np.asarray(src, dtype=np.float32)
    dest = np.asarray(dest, dtype=np.float32)
    edge_attr = np.asarray(edge_attr, dtype=np.float32)
    u = np.asarray(u, dtype=np.float32)
    W1 = np.asarray(W1, dtype=np.float32)
    b1 = np.asarray(b1, dtype=np.float32)
    W2 = np.asarray(W2, dtype=np.float32)
    b2 = np.asarray(b2, dtype=np.float32)
    b = np.asarray(batch).astype(np.int64)

    E, D = src.shape
    G = u.shape[0]
    assert D == P and E % N_CORES == 0
    E0 = E // N_CORES
    Epad = ((E0 + TILE - 1) // TILE) * TILE
    T = Epad // TILE

    # Fold u[batch] @ W1d + b1 into a per-group table (tiny: G x D).
    z = (u @ W1[3 * D:4 * D] + b1).astype(np.float32)  # [G, D]

    # Per-core: tile-local group offsets for the z-selection matmul.
    g0s, js = [], []
    k_s = 1
    for c in range(N_CORES):
        bc = b[c * E0:(c + 1) * E0]
        bp = np.concatenate([bc, np.full(Epad - E0, bc[-1], dtype=np.int64)])
        per_tile = bp.reshape(T, TILE)
        g0 = per_tile.min(axis=1)                 # [T]
        j = bp - np.repeat(g0, TILE)              # [Epad], >= 0
        g0s.append(g0)
        js.append(j)
        k_s = max(k_s, int(j.max()) + 1)

    in_maps = []
    wpk_in = np.concatenate(
        [W1[0 * D:1 * D], W1[1 * D:2 * D], W1[2 * D:3 * D], W2],
        axis=0).reshape(4, D, D).transpose(1, 0, 2).reshape(D, 4 * D)
    wpk_in = np.ascontiguousarray(wpk_in).astype(bf16)
    b2_in = np.ascontiguousarray(b2.reshape(P, 1))
    tile_of_e = np.arange(Epad) // TILE
    for c in range(N_CORES):
        sl = slice(c * E0, (c + 1) * E0)

        def tr(x, dt):
            out = np.zeros((P, Epad), dtype=dt)
            out[:, :E0] = x[sl].T.astype(dt)
            return out

        # one-hot selection rows, strip-packed: tile t's rows at k_s*(t%GRP)
        selc = np.zeros((GRP * k_s, Epad), dtype=e3m4)
        selc[k_s * (tile_of_e % GRP) + js[c], np.arange(Epad)] = 1.0
        selc[:, E0:] = 0.0  # pad edges contribute nothing
        gidx = np.clip(g0s[c][:, None] + np.arange(k_s)[None, :], 0, G - 1)
        zrows = z[gidx]                            # [T, k_s, P]
        zwc = np.zeros((GRP * k_s, T, P), dtype=bf16)
        for m in range(GRP):
            tsel = np.arange(T) % GRP == m
            zwc[k_s * m:k_s * (m + 1), tsel] = \
                zrows[tsel].transpose(1, 0, 2).astype(bf16)
        zwc = np.ascontiguousarray(zwc.reshape(GRP * k_s, T * P))
        in_maps.append({
            "srcT": tr(src, e3m4), "destT": tr(dest, e3m4),
            "eaT": tr(edge_attr, e3m4),
            "wpk": wpk_in, "b2c": b2_in,
            "sel": selc, "zw": zwc,
        })

    profile = os.environ.get("KERNEL_PROFILE", "") == "1"
    if profile:
        _install_profile_shim()

    nc = _get_program(T, k_s)
    from concourse.bass_utils import run_bass_kernel_spmd
    kwargs = {}
    if profile:
        kwargs["trace"] = True
        if os.environ.get("KERNEL_PROFILE_ALL", "") == "1":
            kwargs["trace_cores"] = list(range(N_CORES))
    res = run_bass_kernel_spmd(nc, in_maps, core_ids=list(range(N_CORES)),
                               **kwargs)
    if profile and res.exec_time_ns is not None:
        with open("/tmp/kernel_exec_ns.txt", "w") as f:
            f.write(str(res.exec_time_ns))
        print(f"HW exec time: {res.exec_time_ns} ns")

    out = np.empty((E, P), dtype=np.float32)
    for c in range(N_CORES):
        out[c * E0:(c + 1) * E0] = \
            res.results[c]["outT"][:, :E0].T.astype(np.float32)
    return out
